# revision 17
# baseline (speedup 1.0000x reference)
"""LoftQ linear (4-bit blockwise dequant + linear + LoRA) on 8 trn2 cores.

out = x @ W^T + bias + 2.0 * (x @ A^T) @ B^T
  W[o,i] = (idx[o,i] * 2/15 - 1) * scales[o, i//64]   (idx = 4-bit nibbles)

Sharding: 4 o-shards x 2 t-shards. Each core handles 1024 out_features x
1024 tokens (full contraction 4096). Per-core DMA ~13 MB vs a ~115 us PE
floor, so DMA never gates the matmul stream.

Device kernel (per core):
  - contraction axis i permuted to i' = [even i, odd i]; packed qweight
    bytes ship as-is and are nibble-unpacked on-chip (DVE and/shift),
    so lo/hi nibbles land in the two contiguous halves of each W pair.
  - dequant pipeline per pair k (16 pairs of 128 i'-rows):
    DVE: lo=b&15, hi=b>>4 -> ScalarE: affine c*v-1 (u8->fp16) + u8
    scale->fp16 -> DVE: *scale per half (fp16->bf16). Stage times stay
    under the ~3.5us/pair MM consumption.
  - scales ship as u8 (s*255; 1/255 folded into the ScalarE convert).
  - lora + bias fold into an extra K=17 contraction chunk: host computes
    xa = 2*x@A^T, appends a ones-row; B''=[B^T; bias]. These matmuls are
    inserted after pair 1, off the pipeline-fill critical path.
  - main: 528 matmuls [K,M,N]=[128,128,512] bf16, 8 psum banks = 8
    o-groups; phase 0 accumulates pair-major (follows dequant supply),
    phase 1 is o-major so stores overlap the matmul stream.
  - DMA plan (HWDGE rings serialize transfer + ~3us completion receipt
    per DMA, so head-window loads are fused into few DMAs; SWDGE moves
    bulk but its big packets crowd the rings, so bulk loads are gated):
      sync ring:   lhA(pairs 0-1), lor, lhB(2-5), lhC(6-15)
      scalar ring: hx0/hx1/hx2 (fused scales+x0 for pairs 0-5),
                   stC (after pair 1), outputs
      gpsimd SWDGE: x0C (gated on Wp[1]), x1 (gated on Wp[8])
"""

import numpy as np
import ml_dtypes

OUT_F = 4096
IN_F = 4096
T = 2048
R = 16
NCORES = 8
NOSH = 4  # o-shards
NTSH = 2  # t-shards
O_SH = OUT_F // NOSH  # 1024
T_SH = T // NTSH  # 1024
IPH = IN_F // 2  # 2048 packed byte-rows
C16 = 2.0 / 15.0
NQ = IPH // 128  # 16 pairs
NI = IN_F // 128  # 32 i' chunks
NO = O_SH // 128  # 8 o tiles
NT = T_SH // 512  # 2 t phases
KL = R + 1  # lora+bias contraction rows
NHX = 3  # fused head groups over pairs 0-5, sizes 1/2/3 pairs
HXG = [(0, 1), (1, 2), (3, 3)]  # (first pair, npairs) per head group
HXB = 3 * 4096  # padded bytes/partition (npairs * 4KB used per group)

BF16 = ml_dtypes.bfloat16
FP16 = np.float16

_cached = {}


def _build_nc():
    import concourse.bacc as bacc
    import concourse.mybir as mybir
    from concourse.tile import TileContext

    f32 = mybir.dt.float32
    bf16 = mybir.dt.bfloat16
    fp16 = mybir.dt.float16
    u8 = mybir.dt.uint8
    AF = mybir.ActivationFunctionType
    OP = mybir.AluOpType

    nc = bacc.Bacc("TRN2", target_bir_lowering=False)

    lh = nc.dram_tensor("lh", [128, NQ, O_SH], u8, kind="ExternalInput")
    hx = nc.dram_tensor("hx", [NHX, 128, HXB], u8, kind="ExternalInput")
    st = nc.dram_tensor("st", [128, NQ, O_SH], u8, kind="ExternalInput")
    x0p = nc.dram_tensor("x0p", [128, 2 * NQ, 512], bf16, kind="ExternalInput")
    x1 = nc.dram_tensor("x1", [128, NI, 512], bf16, kind="ExternalInput")
    # lor: [bw (o cols 0:1024) | xab (t chunks 0,1)] as [KL, 4, 512]
    lor = nc.dram_tensor("lor", [KL, 4, 512], bf16, kind="ExternalInput")
    out = nc.dram_tensor("out", [O_SH, T_SH], bf16, kind="ExternalOutput")

    with TileContext(nc) as tc:
        with (
            tc.tile_pool(name="w", bufs=1) as wpool,
            tc.tile_pool(name="x", bufs=1) as xpool,
            tc.tile_pool(name="cst", bufs=1) as cpool,
            tc.tile_pool(name="nib", bufs=2) as nibpool,
            tc.tile_pool(name="dq", bufs=2) as dqpool,
            tc.tile_pool(name="sc", bufs=3) as scpool,
            tc.tile_pool(name="outp", bufs=3) as opool,
            tc.tile_pool(name="ps", bufs=8, space="PSUM") as pspool,
        ):
            lor_sb = cpool.tile([KL, 4, 512], bf16, tag="lor", name="lorsb")

            wsc = cpool.tile([128, 512], bf16, tag="wsc", name="wsc")
            nc.vector.memset(wsc[:], 0)

            # scalar ring: fused lh+scale+x0 groups for pairs 0-5.
            # hx0 is emitted before everything else so its completion lane
            # is not shared with (gated behind) a bigger transfer
            hxt = []
            for gi, (p0_, npr) in enumerate(HXG):
                h_ = cpool.tile(
                    [128, npr * 4096], u8, tag=f"hx{gi}", name=f"hx{gi}"
                )
                hxt.append(h_)
            nc.scalar.dma_start(out=hxt[0][:], in_=hx[0, :, 0:4096])

            # sync ring: lor first (lora warm-up matmuls), then lhC
            nc.sync.dma_start(out=lor_sb[:], in_=lor[:, :, :])
            lhC = cpool.tile([128, 10, O_SH], u8, tag="lhC", name="lhC")
            nc.sync.dma_start(out=lhC[:], in_=lh[:, 6:16, :])

            for gi, (p0_, npr) in enumerate(HXG):
                if gi > 0:
                    nc.scalar.dma_start(
                        out=hxt[gi][:], in_=hx[gi, :, 0 : npr * 4096]
                    )
            stC = cpool.tile([128, 10, O_SH], u8, tag="stC", name="stC")
            x0C = cpool.tile([128, 20, 512], bf16, tag="x0C", name="x0C")
            x1_sb = xpool.tile([128, NI, 512], bf16, tag="x1", name="x1sb")

            Wp = [
                wpool.tile([128, 2 * O_SH], bf16, tag=f"w{k}", name=f"wt{k}")
                for k in range(NQ)
            ]

            ps0 = [
                pspool.tile([128, 512], f32, tag="mm", name=f"p0_{og}")
                for og in range(NO)
            ]

            def lora_mm(ps, og, tcn, start=True):
                # bw col-block: og 0-3 in lor[:,0,:], og 4-7 in lor[:,1,:]
                nc.tensor.matmul(
                    ps[:],
                    lor_sb[:, og // 4, (og % 4) * 128 : (og % 4 + 1) * 128],
                    lor_sb[:, 2 + tcn, :],
                    start=start,
                    stop=False,
                )

            # PE warm-up: dummy matmuls open the HAM clock gate, then the
            # lora+bias matmuls (start=True opens each psum group) bridge
            # until the first weight pair exits the dequant pipeline
            for d in range(12):
                nc.tensor.matmul(
                    ps0[0][:], wsc[:, :128], wsc[:], start=(d == 0), stop=(d == 11)
                )
            for og in range(NO):
                lora_mm(ps0[og], og, 0)

            def pair_mms(k, half, x_ap, stop, start=False):
                for og in range(NO):
                    nc.tensor.matmul(
                        ps0[og][:],
                        Wp[k][
                            :,
                            half * O_SH + og * 128 : half * O_SH + (og + 1) * 128,
                        ],
                        x_ap,
                        start=start,
                        stop=stop,
                    )

            def sconv(st_ap, k):
                # u8 scale row -> fp16 on ScalarE (1/255 folded here)
                s16 = scpool.tile([128, O_SH], fp16, tag="s16", name=f"s16_{k}")
                nc.scalar.activation(
                    s16[:], st_ap, AF.Copy, bias=0.0, scale=1.0 / 255.0
                )
                return s16

            def pair_srcs(k):
                """(lh_ap, st_ap, x0_ap(half)) for pair k."""
                if k < 6:
                    gi = next(i for i, (p, n) in enumerate(HXG) if p <= k < p + n)
                    j, npr = k - HXG[gi][0], HXG[gi][1]
                    lh_ap = hxt[gi][:, j * O_SH : (j + 1) * O_SH]
                    st_ap = hxt[gi][:, (npr + j) * O_SH : (npr + j + 1) * O_SH]

                    def x0_ap(half, gi=gi, j=j, npr=npr):
                        b0 = 2 * npr * O_SH + (2 * j + half) * 1024
                        return hxt[gi][:, b0 : b0 + 1024].bitcast(bf16)

                else:
                    j = k - 6
                    lh_ap = lhC[:, j, :]
                    st_ap = stC[:, j, :]

                    def x0_ap(half, j=j):
                        return x0C[:, 2 * j + half, :]

                return lh_ap, st_ap, x0_ap

            # dequant + phase-0 matmuls, pair-major; pair 0 runs as two
            # half-width chains for fast pipeline fill
            for k in range(NQ):
                lh_ap, st_ap, x0_ap = pair_srcs(k)
                nib = nibpool.tile([128, 2 * O_SH], u8, tag="nib", name=f"nib{k}")
                up = dqpool.tile([128, 2 * O_SH], fp16, tag="up", name=f"up{k}")
                if k == 0:
                    s16 = sconv(st_ap, 0)
                    for half in range(2):
                        hs = slice(half * O_SH, (half + 1) * O_SH)
                        if half == 0:
                            nc.vector.tensor_scalar(
                                nib[:, hs], lh_ap, 15, None, OP.bitwise_and
                            )
                        else:
                            nc.vector.tensor_scalar(
                                nib[:, hs], lh_ap, 4, None, OP.logical_shift_right
                            )
                        nc.scalar.activation(
                            up[:, hs], nib[:, hs], AF.Copy, bias=-1.0, scale=C16
                        )
                        nc.vector.tensor_tensor(
                            Wp[0][:, hs], up[:, hs], s16[:], OP.mult
                        )
                        pair_mms(0, half, x0_ap(half), False)
                    continue
                nc.vector.tensor_scalar(
                    nib[:, :O_SH], lh_ap, 15, None, OP.bitwise_and
                )
                nc.vector.tensor_scalar(
                    nib[:, O_SH:], lh_ap, 4, None, OP.logical_shift_right
                )
                nc.scalar.activation(up[:], nib[:], AF.Copy, bias=-1.0, scale=C16)
                s16 = sconv(st_ap, k)
                for half in range(2):
                    hs = slice(half * O_SH, (half + 1) * O_SH)
                    nc.vector.tensor_tensor(Wp[k][:, hs], up[:, hs], s16[:], OP.mult)
                    pair_mms(
                        k, half, x0_ap(half), stop=(k == NQ - 1 and half == 1)
                    )
                if k == 1:
                    # release the tail scale chunk and (via a scribble dep)
                    # the bulk x0 SWDGE load whose big packets would
                    # otherwise crowd out the weight streams
                    nc.scalar.dma_start(out=stC[:], in_=st[:, 6:16, :])
                    nc.scalar.copy(x0C[:, 0, 0:1], Wp[1][:, 0:1])
                    nc.gpsimd.dma_start(out=x0C[:], in_=x0p[:, 12:32, :])
                if k == 8:
                    # release x1 behind the whole weight stream
                    nc.scalar.copy(x1_sb[:, 0, 0:1], Wp[8][:, 0:1])
                    nc.gpsimd.dma_start(out=x1_sb[:], in_=x1[:, :, :])

            def store(p, tcn, og):
                o_sb = opool.tile([128, 512], bf16, tag="osb", name=f"osb{tcn}_{og}")
                nc.vector.tensor_copy(o_sb[:], p[:])
                nc.scalar.dma_start(
                    out=out[og * 128 : (og + 1) * 128, tcn * 512 : (tcn + 1) * 512],
                    in_=o_sb[:],
                )

            for og in range(NO):
                store(ps0[og], 0, og)

            # phase 1: weights resident -> o-major, stores overlap stream
            for og in range(NO):
                p = pspool.tile([128, 512], f32, tag="mm", name=f"p1_{og}")
                lora_mm(p, og, 1)
                for k in range(NQ):
                    for half in range(2):
                        ic = k + half * NQ
                        nc.tensor.matmul(
                            p[:],
                            Wp[k][
                                :,
                                half * O_SH + og * 128 : half * O_SH + (og + 1) * 128,
                            ],
                            x1_sb[:, ic, :],
                            start=False,
                            stop=(k == NQ - 1 and half == 1),
                        )
                store(p, 1, og)
    nc.compile()
    return nc


def _pack_rows(a, nblk):
    """[nblk*128, F] -> [128, nblk, F] with blk j, partition p = row j*128+p."""
    f = a.shape[1]
    return np.ascontiguousarray(a.reshape(nblk, 128, f).transpose(1, 0, 2))


def prep_inputs(x, qweight, scales, bias, lora_A, lora_B):
    """Host-side layout prep + sharding. Returns per-core input maps."""
    x2d = np.ascontiguousarray(x.reshape(T, IN_F)).astype(np.float32)
    qw2 = np.asarray(qweight, dtype=np.int64).reshape(OUT_F, IPH)
    sc2 = np.asarray(scales, dtype=np.float32).reshape(OUT_F, IN_F // 64)
    bias = np.asarray(bias, dtype=np.float32)
    lora_A = np.asarray(lora_A, dtype=np.float32)
    lora_B = np.asarray(lora_B, dtype=np.float32)

    # per o-shard weight-side tensors
    osh = []
    for s in range(NOSH):
        o0, o1 = s * O_SH, (s + 1) * O_SH
        lh_c = _pack_rows(qw2[o0:o1].T, NQ).astype(np.uint8)  # [128, NQ, O_SH]
        st_c = _pack_rows(
            np.round(np.repeat(sc2[o0:o1].T, 32, axis=0) * 255.0), NQ
        ).astype(np.uint8)
        bw_c = np.concatenate(
            [lora_B[o0:o1].T, bias[None, o0:o1]], axis=0
        ).astype(np.float32)  # [17, O_SH]
        osh.append((lh_c, st_c, bw_c))

    # per t-shard x-side tensors
    tsh = []
    for t in range(NTSH):
        t0, t1 = t * T_SH, (t + 1) * T_SH
        xt = x2d[t0:t1].T  # [IN_F, T_SH]
        xp = np.concatenate([xt[0::2], xt[1::2]], axis=0)  # i' permutation
        xb = _pack_rows(xp, NI)  # [128, NI, T_SH]
        xb = xb.reshape(128, NI, NT, 512)
        x1_c = np.ascontiguousarray(xb[:, :, 1, :]).astype(BF16)  # [128, NI, 512]
        x0n = xb[:, :, 0, :]  # [128, NI, 512] natural chunk order
        x0_order = []
        for k in range(NQ):
            x0_order += [k, NQ + k]
        x0_c = np.ascontiguousarray(x0n[:, x0_order, :]).astype(BF16)
        xa = 2.0 * (x2d[t0:t1] @ lora_A.T)  # [T_SH, R]
        xab_c = np.concatenate([xa.T, np.ones((1, T_SH), np.float32)], axis=0)
        tsh.append((x0_c, x1_c, xab_c))

    in_maps = []
    for c in range(NCORES):
        s, t = c // NTSH, c % NTSH
        lh_c, st_c, bw_c = osh[s]
        x0_c, x1_c, xab_c = tsh[t]
        lor_c = np.concatenate([bw_c, xab_c], axis=1)  # [17, 2048]
        lor_c = np.ascontiguousarray(lor_c.reshape(KL, 4, 512)).astype(BF16)
        # fused head groups: lh rows + scale rows + x0 slices per group
        hx_c = np.zeros((NHX, 128, HXB), dtype=np.uint8)
        for gi, (p0_, npr) in enumerate(HXG):
            lhg = lh_c[:, p0_ : p0_ + npr, :].reshape(128, npr * O_SH)
            stg = st_c[:, p0_ : p0_ + npr, :].reshape(128, npr * O_SH)
            x0g = (
                np.ascontiguousarray(x0_c[:, 2 * p0_ : 2 * (p0_ + npr), :])
                .view(np.uint8)
                .reshape(128, npr * 2048)
            )
            hx_c[gi, :, 0 : npr * 4096] = np.concatenate([lhg, stg, x0g], axis=1)
        in_maps.append(
            {
                "lh": lh_c,
                "st": st_c,
                "hx": np.ascontiguousarray(hx_c),
                "x0p": x0_c,
                "x1": x1_c,
                "lor": lor_c,
            }
        )
    return in_maps


def run(in_maps, trace=False):
    from concourse import bass_utils

    if "nc" not in _cached:
        _cached["nc"] = _build_nc()
    res = bass_utils.run_bass_kernel_spmd(
        _cached["nc"], in_maps, list(range(NCORES)), trace=trace
    )
    return res


def assemble(results):
    full = np.zeros((OUT_F, T), dtype=np.float32)
    for c, r in enumerate(results):
        s, t = c // NTSH, c % NTSH
        full[s * O_SH : (s + 1) * O_SH, t * T_SH : (t + 1) * T_SH] = np.asarray(
            r["out"], dtype=np.float32
        )
    return np.ascontiguousarray(full.T).reshape(2, 1024, OUT_F)


def kernel(x, qweight, scales, bias, lora_A, lora_B):
    in_maps = prep_inputs(x, qweight, scales, bias, lora_A, lora_B)
    res = run(in_maps, trace=False)
    return assemble(res.results)
